# revision 57
# baseline (speedup 1.0000x reference)
"""GCNConv mean-aggregation kernel for 8 Trainium2 NeuronCores.

Reference computation:
    msgs   = x[src]                       # [E, D] gather
    summed = segment_sum(msgs, dst, N)    # [N, D]
    deg    = segment_sum(ones, dst, N)    # [N]
    h      = summed / max(deg, 1)
    out    = h @ W.T + b                  # [N, D_OUT]

Strategy (no collectives needed):
  - Nodes are grouped into 392 windows of 128 dst nodes.  Windows are
    sorted by edge count and dealt rank-adjacent to the 8 cores so the
    SPMD max-over-core subtile schedule has minimal padding; each core
    fully reduces the 49 windows it owns and the host undoes the
    permutation when reassembling.
  - The per-class edge streams are gathered from HBM with dma_gather
    (256B bf16 rows: 64 feats + zero pad) in 1024-index per-packet
    calls rotated over the 4 SWDGE queues; padding rows gather a zero
    row and carry dst_rel=-1 so they match no one-hot column.
  - The dst-relative values are expanded across the free dim with a
    K=CHUNK PE matmul against a block-identity (rep[p, s*128+w] =
    dst(s, p)), cast f32->bf16 on the ACT engine, and compared to an
    iota on DVE (packed bf16 operands keep the is_equal in the 2x
    single-port mode, which also avoids starving SWDGE descriptor
    writes).  The one-hot matmul-accumulates onehot.T @ msgs into a
    [128, 64] PSUM tile per window.
  - Degrees come from a host-side bincount over dst (index-only work);
    the kernel multiplies each window's PSUM sums by a per-node
    reciprocal table (ACT), transposes h via the PE identity trick,
    applies W (as lhsT = W.T, bf16) and bias, and writes out.T slices
    ([64, 6272] per core).  Host reassembles/permutes/transposes.
  - dma_gather indices are int16, so x is staged into two gather tables
    (src < 32767 and src >= 32767), each with a zero row at index 0.
"""

import sys

sys.path.insert(0, "/opt/trn_rl_repo")

import numpy as np
import ml_dtypes

import concourse.bacc as bacc
import concourse.mybir as mybir
import concourse.tile as tile
from concourse.bass_utils import run_bass_kernel_spmd

N_NODES = 50000
N_EDGES = 800000
D = 64
N_CORES = 8
NPC = 6272          # nodes per core (= 49 windows of 128)
WIN = 128           # dst-window width per PSUM accumulation group
N_WIN = NPC // WIN  # 49
N_GWIN = N_CORES * N_WIN  # 392 global windows
SPLIT = 32767       # src < SPLIT -> lo table, else hi table
ROW = 128           # gather row elems (bf16): 64 feats + zero pad = 256 B
CHUNK = 16          # max subtiles (of 128 edges) per dma_gather call
NQ = 4              # SWDGE queues for parallel gather descriptor work

F32 = mybir.dt.float32
BF16 = mybir.dt.bfloat16
I16 = mybir.dt.int16
I32 = mybir.dt.int32
BF = ml_dtypes.bfloat16

# Results of the most recent run (for test harness inspection).
LAST = {}


def _schedule(kA, kB, offA, offB):
    """Shared SPMD call list: full-CHUNK runs of each class's global
    subtile stream (class A first, then class B).  Returns
    [(st, stream_pos, nsub)]."""
    SA = int(kA.sum())
    SBp = max(int(kB.sum()), 1)
    calls = [(0, p, min(CHUNK, SA - p)) for p in range(0, SA, CHUNK)]
    calls += [(1, p, min(CHUNK, SBp - p)) for p in range(0, SBp, CHUNK)]
    return calls


def _prep(x, src, dst):
    """Host-side sharding: build bf16 gather tables, reciprocal-degree
    tables, the balanced window->core assignment, per-core padded edge
    streams (int16 gather idx + bf16 dst-rel + per-call real counts),
    and the shared subtile schedule."""
    x = np.asarray(x, dtype=np.float32)
    src = np.asarray(src, dtype=np.int64)
    dst = np.asarray(dst, dtype=np.int64)

    n_lo = SPLIT
    n_hi = N_NODES - SPLIT
    xlo = np.zeros((n_lo + 1, ROW), dtype=BF)
    xlo[1:, :D] = x[:SPLIT].astype(BF)
    xhi = np.zeros((n_hi + 1, ROW), dtype=BF)
    xhi[1:, :D] = x[SPLIT:].astype(BF)

    # Reciprocal degree per node (padded range), from indices only.
    deg = np.bincount(dst, minlength=N_GWIN * WIN).astype(np.float64)
    rec_all = (1.0 / np.maximum(deg, 1.0)).astype(np.float32)

    # Balanced assignment: sort global windows by edge count, deal the
    # 8 rank-adjacent windows of each group to the 8 cores.
    gw = (dst // WIN).astype(np.int64)
    wcnt = np.bincount(gw, minlength=N_GWIN)
    order_w = np.argsort(-wcnt, kind="stable")
    assign = np.empty((N_CORES, N_WIN), dtype=np.int64)
    for j in range(N_WIN):
        assign[:, j] = order_w[j * N_CORES : (j + 1) * N_CORES]
    core_of = np.empty(N_GWIN, dtype=np.int64)
    slot_of = np.empty(N_GWIN, dtype=np.int64)
    for c in range(N_CORES):
        for j in range(N_WIN):
            core_of[assign[c, j]] = c
            slot_of[assign[c, j]] = j

    cls = (src >= SPLIT).astype(np.int64)
    key = (core_of[gw] * N_WIN + slot_of[gw]) * 2 + cls
    order = np.argsort(key, kind="stable")
    src_s, dst_s = src[order], dst[order]

    n_groups = (N_CORES * N_WIN) * 2
    counts = np.bincount(key[order], minlength=n_groups)
    starts = np.zeros(n_groups + 1, dtype=np.int64)
    np.cumsum(counts, out=starts[1:])

    cnt = counts.reshape(N_CORES, N_WIN, 2)
    kA = np.maximum(1, -(-cnt[:, :, 0].max(axis=0) // 128))  # [N_WIN]
    kB = -(-cnt[:, :, 1].max(axis=0) // 128)                  # [N_WIN]
    SA = int(kA.sum())
    SB = int(kB.sum())
    SBp = max(SB, 1)

    idx_lo = (src_s + 1).astype(np.int16)
    idx_hi = (src_s - SPLIT + 1).astype(np.int16)

    offA = np.zeros(N_WIN + 1, dtype=np.int64)
    np.cumsum(kA, out=offA[1:])
    offB = np.zeros(N_WIN + 1, dtype=np.int64)
    np.cumsum(kB, out=offB[1:])

    gw_s = (dst_s // WIN).astype(np.int64)
    base_s = (gw_s * WIN).astype(np.int64)
    drel_s = (dst_s - base_s).astype(BF)

    calls = _schedule(kA, kB, offA, offB)
    ncalls = len(calls)

    per_core = []
    for c in range(N_CORES):
        iA = np.zeros(SA * 128, dtype=np.int16)
        iB = np.zeros(SBp * 128, dtype=np.int16)
        dA = np.full(SA * 128, -1.0, dtype=BF)
        dB = np.full(SBp * 128, -1.0, dtype=BF)
        realA = np.zeros(N_WIN, dtype=np.int64)
        realB = np.zeros(N_WIN, dtype=np.int64)
        for j in range(N_WIN):
            g = (c * N_WIN + j) * 2
            s0, s1 = starts[g], starts[g + 1]
            p0 = int(offA[j]) * 128
            iA[p0 : p0 + (s1 - s0)] = idx_lo[s0:s1]
            dA[p0 : p0 + (s1 - s0)] = drel_s[s0:s1]
            realA[j] = s1 - s0
            s0, s1 = starts[g + 1], starts[g + 2]
            p0 = int(offB[j]) * 128
            iB[p0 : p0 + (s1 - s0)] = idx_hi[s0:s1]
            dB[p0 : p0 + (s1 - s0)] = drel_s[s0:s1]
            realB[j] = s1 - s0
        # per-call real-index counts (>=1: degenerate calls keep one
        # zero-row index so the gather has a valid trailing position)
        cnts = np.zeros(ncalls, dtype=np.int32)
        woffA = {int(offA[j]): j for j in range(N_WIN)}
        woffB = {int(offB[j]): j for j in range(N_WIN)}
        posA = {int(offA[j]) + p: j for j in range(N_WIN) for p in range(int(kA[j]))}
        posB = {int(offB[j]) + p: j for j in range(N_WIN) for p in range(int(kB[j]))}
        for i, (st, pos, nsub) in enumerate(calls):
            if st == 0:
                j = posA[pos]
                real = int(realA[j]) - (pos - int(offA[j])) * 128
                istream = iA
            else:
                j = posB[pos]
                real = int(realB[j]) - (pos - int(offB[j])) * 128
                istream = iB
            real = max(0, min(real, nsub * 128))
            if real == 0:
                istream[pos * 128] = 0  # zero row; dst_rel stays -1
                real = 1
            cnts[i] = real
        rec_c = np.empty((WIN, N_WIN), dtype=np.float32)
        for j in range(N_WIN):
            g0 = assign[c, j] * WIN
            rec_c[:, j] = rec_all[g0 : g0 + WIN]
        per_core.append((iA, dA, iB, dB, cnts, rec_c))

    return xlo, xhi, kA, kB, SA, SBp, offA, offB, calls, per_core, assign


def _wrap_idx(idx_flat):
    """int16 stream -> dma_gather layout [128, n/16]: value i at
    [i % 16, i // 16], replicated across the 8 groups of 16 partitions."""
    a = idx_flat.reshape(-1, 16).T
    return np.tile(a, (8, 1)).copy()


def _wrap_dst(dA, dB, calls):
    """bf16 class streams -> [CHUNK, ncalls*128]: call i subtile s lane e
    at [s, i*128 + e], the lhsT of the per-chunk replication matmul."""
    out = np.zeros((CHUNK, len(calls) * 128), dtype=BF)
    for i, (st, pos, nsub) in enumerate(calls):
        d = dA if st == 0 else dB
        for s in range(nsub):
            out[s, i * 128 : (i + 1) * 128] = d[
                (pos + s) * 128 : (pos + s + 1) * 128
            ]
    return out


def _build_program(kA, kB, SA, SBp, offA, offB, calls):
    nc = bacc.Bacc(
        "TRN2", target_bir_lowering=False, debug=False, num_swdge_queues=NQ
    )
    ncalls = len(calls)

    t_xlo = nc.dram_tensor("xlo", [SPLIT + 1, ROW], BF16, kind="ExternalInput")
    t_xhi = nc.dram_tensor(
        "xhi", [N_NODES - SPLIT + 1, ROW], BF16, kind="ExternalInput"
    )
    t_wt = nc.dram_tensor("wt", [D, D], BF16, kind="ExternalInput")
    t_b = nc.dram_tensor("bias", [D, 1], F32, kind="ExternalInput")
    t_ia = nc.dram_tensor("idxa", [128, SA * 8], I16, kind="ExternalInput")
    t_ib = nc.dram_tensor("idxb", [128, SBp * 8], I16, kind="ExternalInput")
    t_dd = nc.dram_tensor("dstt", [CHUNK, ncalls * 128], BF16, kind="ExternalInput")
    t_ones = nc.dram_tensor("ones", [CHUNK, CHUNK * 128], BF16, kind="ExternalInput")
    t_iota = nc.dram_tensor("iota", [128, CHUNK * WIN], BF16, kind="ExternalInput")
    t_rec = nc.dram_tensor("rec", [WIN, N_WIN], F32, kind="ExternalInput")
    t_id = nc.dram_tensor("ident", [WIN, WIN], BF16, kind="ExternalInput")
    t_out = nc.dram_tensor("out", [D, NPC], F32, kind="ExternalOutput")

    ncallsA = -(-SA // CHUNK)

    NSLICE = 6

    with tile.TileContext(nc) as tc:
        with (
            tc.tile_pool(name="const", bufs=1) as cpool,
            tc.tile_pool(name="idx", bufs=1) as ipool,
            tc.tile_pool(name="msgsa", bufs=3) as mpa,
            tc.tile_pool(name="msgsb", bufs=2) as mpb,
            tc.tile_pool(name="oha", bufs=3) as opa,
            tc.tile_pool(name="ohb", bufs=2) as opb,
            tc.tile_pool(name="norm", bufs=4) as npool,
            tc.tile_pool(name="hpo", bufs=2) as hpool,
            tc.tile_pool(name="repb", bufs=3) as rep_pool,
            tc.tile_pool(name="psacc", bufs=3, space="PSUM") as ps_acc,
            tc.tile_pool(name="pstr", bufs=2, space="PSUM") as ps_tr,
            tc.tile_pool(name="psz", bufs=1, space="PSUM") as ps_z,
            tc.tile_pool(name="psrep", bufs=2, space="PSUM") as ps_rep,
        ):
            # ---- constants (iota / identity supplied from host) ----
            ident = cpool.tile([WIN, WIN], BF16)
            nc.sync.dma_start(out=ident[:], in_=t_id[:])
            wt_sb = cpool.tile([D, D], BF16)
            nc.sync.dma_start(out=wt_sb[:], in_=t_wt[:])
            b_sb = cpool.tile([D, 1], F32)
            nc.sync.dma_start(out=b_sb[:], in_=t_b[:])
            iota_f = cpool.tile([128, CHUNK * WIN], BF16)
            nc.sync.dma_start(out=iota_f[:], in_=t_iota[:])
            rec_sb = cpool.tile([WIN, N_WIN], F32)
            nc.sync.dma_start(out=rec_sb[:], in_=t_rec[:])
            ones_sb = cpool.tile([CHUNK, CHUNK * 128], BF16)
            nc.sync.dma_start(out=ones_sb[:], in_=t_ones[:])

            # index / dst tables, loaded in slices so the first gathers
            # can start before the whole tables arrive
            # warmup: a minimal gather issued first so the Q7 ext-isa
            # IRAM load (~6us) overlaps the index-table DMAs; the zero
            # indices come from a local memset, not a DMA
            wu_idx = cpool.tile([128, 8], I16)
            nc.vector.memset(wu_idx[:], 0)
            wu_out = cpool.tile([128, 1, ROW], BF16)
            nc.gpsimd.dma_gather(
                wu_out[:, :1, :], t_xlo[:], wu_idx[:, :8], 128, 128, ROW,
                single_packet=False, queue_num=0,
            )

            ia_sb = ipool.tile([128, SA * 8], I16)
            ib_sb = ipool.tile([128, SBp * 8], I16)
            dd_sb = ipool.tile([CHUNK, ncalls * 128], BF16)

            def sliced_load(sb, t, width, unit):
                n = -(-width // unit)
                first = max(1, n // 16) * unit
                per = -(-(width - first) // (NSLICE * unit)) * unit
                bounds = [0, first]
                while bounds[-1] < width:
                    bounds.append(min(bounds[-1] + per, width))
                for a, b in zip(bounds, bounds[1:]):
                    nc.sync.dma_start(out=sb[:, a:b], in_=t[:, a:b])

            sliced_load(ia_sb, t_ia, SA * 8, 8)
            sliced_load(ib_sb, t_ib, SBp * 8, 8)
            sliced_load(dd_sb, t_dd, ncalls * 128, 128)

            out_sb = cpool.tile([D, NPC], F32)

            call_tiles = {}
            cursor = [0, 0]

            def emit_call(i):
                st, pos, nsub = calls[i]
                if st == 0:
                    mp, op, tsrc, isb = mpa, opa, t_xlo, ia_sb
                else:
                    mp, op, tsrc, isb = mpb, opb, t_xhi, ib_sb
                msgs = mp.tile([128, CHUNK, ROW], BF16)
                nidx = nsub * 128
                # per-packet mode: one SDMA packet per descriptor.
                # single_packet=True measures the same speed here but
                # intermittently corrupts gathers; per-packet is the
                # reliable mode.  Padding rows gather the zero row.
                nc.gpsimd.dma_gather(
                    msgs[:, :nsub, :],
                    tsrc[:],
                    isb[:, pos * 8 : pos * 8 + nsub * 8],
                    nidx,
                    nidx,
                    ROW,
                    single_packet=False,
                    queue_num=i % NQ,
                )
                # Expand this call's dst values along the free dim with
                # K=CHUNK PE matmuls (lhsT = dst chunk [CHUNK subtiles, 128
                # lanes], rhs = block-identity ones -> rep[p, s*128+w] =
                # dst(s, p)), cast f32 PSUM -> bf16 SBUF on the idle ACT
                # engine, then compare on DVE.  The packed bf16 replica
                # (instead of a stride-0 broadcast operand) keeps the
                # is_equal in the 2x single-port DVE mode, which also
                # avoids starving SWDGE descriptor writes.
                rep_bf = rep_pool.tile([128, CHUNK * WIN], BF16)
                half = 512  # one PSUM bank of f32 per matmul
                for h in range(CHUNK * 128 // half):
                    c0 = h * half
                    n_h = min(nsub * 128 - c0, half)
                    if n_h <= 0:
                        break
                    rep = ps_rep.tile([128, half], F32)
                    nc.tensor.matmul(
                        out=rep[:, :n_h],
                        lhsT=dd_sb[:, i * 128 : (i + 1) * 128],
                        rhs=ones_sb[:, c0 : c0 + n_h],
                        start=True,
                        stop=True,
                    )
                    nc.scalar.activation(
                        out=rep_bf[:, c0 : c0 + n_h],
                        in_=rep[:, :n_h],
                        func=mybir.ActivationFunctionType.Copy,
                    )
                oh = op.tile([128, CHUNK * WIN], BF16)
                nc.vector.tensor_tensor(
                    out=oh[:, : nsub * WIN],
                    in0=iota_f[:, : nsub * WIN],
                    in1=rep_bf[:, : nsub * WIN],
                    op=mybir.AluOpType.is_equal,
                )
                call_tiles[i] = (msgs, oh)

            def tiles_for(st, s):
                k = s // CHUNK
                while cursor[st] <= k:
                    emit_call(cursor[st] if st == 0 else ncallsA + cursor[st])
                    cursor[st] += 1
                i = k if st == 0 else ncallsA + k
                msgs, oh = call_tiles[i]
                return msgs, oh, s % CHUNK

            OUT_PIECES = 4
            opiece = [0]

            for w in range(N_WIN):
                subs = [(0, int(offA[w]) + j) for j in range(int(kA[w]))]
                subs += [(1, int(offB[w]) + j) for j in range(int(kB[w]))]
                ps = ps_acc.tile([WIN, D], F32)
                for j, (st, s) in enumerate(subs):
                    msgs, oh, col = tiles_for(st, s)
                    nc.tensor.matmul(
                        out=ps[:],
                        lhsT=oh[:, col * WIN : (col + 1) * WIN],
                        rhs=msgs[:, col, :D],
                        start=(j == 0),
                        stop=(j == len(subs) - 1),
                    )
                h_w = npool.tile([WIN, D], BF16)
                nc.scalar.activation(
                    out=h_w[:],
                    in_=ps[:],
                    func=mybir.ActivationFunctionType.Copy,
                    scale=rec_sb[:, w : w + 1],
                )
                pst = ps_tr.tile([D, WIN], BF16)
                nc.tensor.transpose(out=pst[:], in_=h_w[:], identity=ident[:])
                ht = hpool.tile([D, WIN], BF16)
                nc.scalar.activation(
                    out=ht[:], in_=pst[:], func=mybir.ActivationFunctionType.Copy
                )
                z = ps_z.tile([D, WIN], F32)
                nc.tensor.matmul(
                    out=z[:], lhsT=wt_sb[:], rhs=ht[:], start=True, stop=True
                )
                t0 = w * WIN
                nc.scalar.activation(
                    out=out_sb[:, t0 : t0 + WIN],
                    in_=z[:],
                    func=mybir.ActivationFunctionType.Identity,
                    bias=b_sb[:, 0:1],
                )
                # stream the finished out columns back in pieces
                nxt = (opiece[0] + 1) * N_WIN // OUT_PIECES
                if w + 1 == nxt:
                    a = opiece[0] * N_WIN // OUT_PIECES * WIN
                    bnd = nxt * WIN
                    nc.sync.dma_start(out=t_out[:, a:bnd], in_=out_sb[:, a:bnd])
                    opiece[0] += 1

    nc.compile()
    return nc


def kernel(x, src, dst, W, b):
    x = np.asarray(x, dtype=np.float32)
    W = np.asarray(W, dtype=np.float32)
    b = np.asarray(b, dtype=np.float32)

    (xlo, xhi, kA, kB, SA, SBp, offA, offB, calls, per_core, assign) = _prep(
        x, src, dst
    )
    nc = _build_program(kA, kB, SA, SBp, offA, offB, calls)

    wt = np.ascontiguousarray(W.T.astype(BF))
    bcol = np.ascontiguousarray(b.reshape(D, 1))
    iota_arr = np.tile(
        np.arange(WIN, dtype=BF)[None, :], (128, CHUNK)
    ).copy()
    ident_arr = np.eye(WIN, dtype=BF)
    ones_arr = np.zeros((CHUNK, CHUNK * 128), dtype=BF)
    for s in range(CHUNK):
        ones_arr[s, s * 128 : (s + 1) * 128] = 1.0

    in_maps = []
    for c in range(N_CORES):
        iA, dA, iB, dB, cnts, rec_c = per_core[c]
        in_maps.append(
            {
                "xlo": xlo,
                "xhi": xhi,
                "wt": wt,
                "bias": bcol,
                "idxa": _wrap_idx(iA),
                "idxb": _wrap_idx(iB),
                "dstt": _wrap_dst(dA, dB, calls),
                "iota": iota_arr,
                "rec": rec_c,
                "ident": ident_arr,
                "ones": ones_arr,
            }
        )

    res = run_bass_kernel_spmd(nc, in_maps, list(range(N_CORES)))
    LAST["results"] = res
    LAST["exec_time_ns"] = res.exec_time_ns

    # Undo the balanced window permutation: core c slot j holds global
    # window assign[c, j] as out columns [j*WIN, (j+1)*WIN).
    full = np.empty((N_GWIN * WIN, D), dtype=np.float32)
    for c in range(N_CORES):
        out_c = res.results[c]["out"]  # [D, NPC]
        for j in range(N_WIN):
            g = int(assign[c, j])
            full[g * WIN : (g + 1) * WIN] = out_c[:, j * WIN : (j + 1) * WIN].T
    return np.ascontiguousarray(full[:N_NODES])


# revision 58
# speedup vs baseline: 1.3523x; 1.3523x over previous
"""GCNConv mean-aggregation kernel for 8 Trainium2 NeuronCores.

Reference computation:
    msgs   = x[src]                       # [E, D] gather
    summed = segment_sum(msgs, dst, N)    # [N, D]
    deg    = segment_sum(ones, dst, N)    # [N]
    h      = summed / max(deg, 1)
    out    = h @ W.T + b                  # [N, D_OUT]

Strategy (no collectives needed):
  - Nodes are grouped into 392 windows of 128 dst nodes.  Windows are
    sorted by edge count and dealt rank-adjacent to the 8 cores so the
    SPMD max-over-core subtile schedule has minimal padding; each core
    fully reduces the 49 windows it owns and the host undoes the
    permutation when reassembling.
  - The per-class edge streams are gathered from HBM with dma_gather
    (256B bf16 rows: 64 feats + zero pad) in 1024-index per-packet
    calls rotated over the 4 SWDGE queues; padding rows gather a zero
    row and carry dst_rel=-1 so they match no one-hot column.
  - The dst-relative values are expanded across the free dim with a
    K=CHUNK PE matmul against a block-identity (rep[p, s*128+w] =
    dst(s, p)), cast f32->bf16 on the ACT engine, and compared to an
    iota on DVE (packed bf16 operands keep the is_equal in the 2x
    single-port mode, which also avoids starving SWDGE descriptor
    writes).  The one-hot matmul-accumulates onehot.T @ msgs into a
    [128, 64] PSUM tile per window.
  - Degrees come from a host-side bincount over dst (index-only work);
    the kernel multiplies each window's PSUM sums by a per-node
    reciprocal table (ACT), transposes h via the PE identity trick,
    applies W (as lhsT = W.T, bf16) and bias, and writes out.T slices
    ([64, 6272] per core).  Host reassembles/permutes/transposes.
  - dma_gather indices are int16, so x is staged into two gather tables
    (src < 32767 and src >= 32767), each with a zero row at index 0.
"""

import sys

sys.path.insert(0, "/opt/trn_rl_repo")

import numpy as np
import ml_dtypes

import concourse.bacc as bacc
import concourse.mybir as mybir
import concourse.tile as tile
from concourse.bass_utils import run_bass_kernel_spmd

N_NODES = 50000
N_EDGES = 800000
D = 64
N_CORES = 8
NPC = 6272          # nodes per core (= 49 windows of 128)
WIN = 128           # dst-window width per PSUM accumulation group
N_WIN = NPC // WIN  # 49
N_GWIN = N_CORES * N_WIN  # 392 global windows
SPLIT = 32767       # src < SPLIT -> lo table, else hi table
ROW = 128           # gather row elems (bf16): 64 feats + zero pad = 256 B
CHUNK = 8           # max subtiles (of 128 edges) per dma_gather call
NQ = 4              # SWDGE queues for parallel gather descriptor work

F32 = mybir.dt.float32
BF16 = mybir.dt.bfloat16
I16 = mybir.dt.int16
I32 = mybir.dt.int32
BF = ml_dtypes.bfloat16

# Results of the most recent run (for test harness inspection).
LAST = {}


def _schedule(kA, kB, offA, offB):
    """Shared SPMD call list: full-CHUNK runs of each class's global
    subtile stream (class A first, then class B).  Returns
    [(st, stream_pos, nsub)]."""
    SA = int(kA.sum())
    SBp = max(int(kB.sum()), 1)
    calls = [(0, p, min(CHUNK, SA - p)) for p in range(0, SA, CHUNK)]
    calls += [(1, p, min(CHUNK, SBp - p)) for p in range(0, SBp, CHUNK)]
    return calls


def _prep(x, src, dst):
    """Host-side sharding: build bf16 gather tables, reciprocal-degree
    tables, the balanced window->core assignment, per-core padded edge
    streams (int16 gather idx + bf16 dst-rel + per-call real counts),
    and the shared subtile schedule."""
    x = np.asarray(x, dtype=np.float32)
    src = np.asarray(src, dtype=np.int64)
    dst = np.asarray(dst, dtype=np.int64)

    n_lo = SPLIT
    n_hi = N_NODES - SPLIT
    xlo = np.zeros((n_lo + 1, ROW), dtype=BF)
    xlo[1:, :D] = x[:SPLIT].astype(BF)
    xhi = np.zeros((n_hi + 1, ROW), dtype=BF)
    xhi[1:, :D] = x[SPLIT:].astype(BF)

    # Reciprocal degree per node (padded range), from indices only.
    deg = np.bincount(dst, minlength=N_GWIN * WIN).astype(np.float64)
    rec_all = (1.0 / np.maximum(deg, 1.0)).astype(np.float32)

    # Balanced assignment: sort global windows by edge count, deal the
    # 8 rank-adjacent windows of each group to the 8 cores.
    gw = (dst // WIN).astype(np.int64)
    wcnt = np.bincount(gw, minlength=N_GWIN)
    order_w = np.argsort(-wcnt, kind="stable")
    assign = np.empty((N_CORES, N_WIN), dtype=np.int64)
    for j in range(N_WIN):
        assign[:, j] = order_w[j * N_CORES : (j + 1) * N_CORES]
    core_of = np.empty(N_GWIN, dtype=np.int64)
    slot_of = np.empty(N_GWIN, dtype=np.int64)
    for c in range(N_CORES):
        for j in range(N_WIN):
            core_of[assign[c, j]] = c
            slot_of[assign[c, j]] = j

    cls = (src >= SPLIT).astype(np.int64)
    key = (core_of[gw] * N_WIN + slot_of[gw]) * 2 + cls
    order = np.argsort(key, kind="stable")
    src_s, dst_s = src[order], dst[order]

    n_groups = (N_CORES * N_WIN) * 2
    counts = np.bincount(key[order], minlength=n_groups)
    starts = np.zeros(n_groups + 1, dtype=np.int64)
    np.cumsum(counts, out=starts[1:])

    cnt = counts.reshape(N_CORES, N_WIN, 2)
    kA = np.maximum(1, -(-cnt[:, :, 0].max(axis=0) // 128))  # [N_WIN]
    kB = -(-cnt[:, :, 1].max(axis=0) // 128)                  # [N_WIN]
    SA = int(kA.sum())
    SB = int(kB.sum())
    SBp = max(SB, 1)

    idx_lo = (src_s + 1).astype(np.int16)
    idx_hi = (src_s - SPLIT + 1).astype(np.int16)

    offA = np.zeros(N_WIN + 1, dtype=np.int64)
    np.cumsum(kA, out=offA[1:])
    offB = np.zeros(N_WIN + 1, dtype=np.int64)
    np.cumsum(kB, out=offB[1:])

    gw_s = (dst_s // WIN).astype(np.int64)
    base_s = (gw_s * WIN).astype(np.int64)
    drel_s = (dst_s - base_s).astype(BF)

    calls = _schedule(kA, kB, offA, offB)
    ncalls = len(calls)

    per_core = []
    for c in range(N_CORES):
        iA = np.zeros(SA * 128, dtype=np.int16)
        iB = np.zeros(SBp * 128, dtype=np.int16)
        dA = np.full(SA * 128, -1.0, dtype=BF)
        dB = np.full(SBp * 128, -1.0, dtype=BF)
        realA = np.zeros(N_WIN, dtype=np.int64)
        realB = np.zeros(N_WIN, dtype=np.int64)
        for j in range(N_WIN):
            g = (c * N_WIN + j) * 2
            s0, s1 = starts[g], starts[g + 1]
            p0 = int(offA[j]) * 128
            iA[p0 : p0 + (s1 - s0)] = idx_lo[s0:s1]
            dA[p0 : p0 + (s1 - s0)] = drel_s[s0:s1]
            realA[j] = s1 - s0
            s0, s1 = starts[g + 1], starts[g + 2]
            p0 = int(offB[j]) * 128
            iB[p0 : p0 + (s1 - s0)] = idx_hi[s0:s1]
            dB[p0 : p0 + (s1 - s0)] = drel_s[s0:s1]
            realB[j] = s1 - s0
        # per-call real-index counts (>=1: degenerate calls keep one
        # zero-row index so the gather has a valid trailing position)
        cnts = np.zeros(ncalls, dtype=np.int32)
        woffA = {int(offA[j]): j for j in range(N_WIN)}
        woffB = {int(offB[j]): j for j in range(N_WIN)}
        posA = {int(offA[j]) + p: j for j in range(N_WIN) for p in range(int(kA[j]))}
        posB = {int(offB[j]) + p: j for j in range(N_WIN) for p in range(int(kB[j]))}
        for i, (st, pos, nsub) in enumerate(calls):
            if st == 0:
                j = posA[pos]
                real = int(realA[j]) - (pos - int(offA[j])) * 128
                istream = iA
            else:
                j = posB[pos]
                real = int(realB[j]) - (pos - int(offB[j])) * 128
                istream = iB
            real = max(0, min(real, nsub * 128))
            if real == 0:
                istream[pos * 128] = 0  # zero row; dst_rel stays -1
                real = 1
            cnts[i] = real
        rec_c = np.empty((WIN, N_WIN), dtype=np.float32)
        for j in range(N_WIN):
            g0 = assign[c, j] * WIN
            rec_c[:, j] = rec_all[g0 : g0 + WIN]
        per_core.append((iA, dA, iB, dB, cnts, rec_c))

    return xlo, xhi, kA, kB, SA, SBp, offA, offB, calls, per_core, assign


def _wrap_idx(idx_flat):
    """int16 stream -> dma_gather layout [128, n/16]: value i at
    [i % 16, i // 16], replicated across the 8 groups of 16 partitions."""
    a = idx_flat.reshape(-1, 16).T
    return np.tile(a, (8, 1)).copy()


def _wrap_dst(dA, dB, calls):
    """bf16 class streams -> [CHUNK, ncalls*128]: call i subtile s lane e
    at [s, i*128 + e], the lhsT of the per-chunk replication matmul."""
    out = np.zeros((CHUNK, len(calls) * 128), dtype=BF)
    for i, (st, pos, nsub) in enumerate(calls):
        d = dA if st == 0 else dB
        for s in range(nsub):
            out[s, i * 128 : (i + 1) * 128] = d[
                (pos + s) * 128 : (pos + s + 1) * 128
            ]
    return out


def _build_program(kA, kB, SA, SBp, offA, offB, calls):
    nc = bacc.Bacc(
        "TRN2", target_bir_lowering=False, debug=False, num_swdge_queues=NQ
    )
    ncalls = len(calls)

    t_xlo = nc.dram_tensor("xlo", [SPLIT + 1, ROW], BF16, kind="ExternalInput")
    t_xhi = nc.dram_tensor(
        "xhi", [N_NODES - SPLIT + 1, ROW], BF16, kind="ExternalInput"
    )
    t_wt = nc.dram_tensor("wt", [D, D], BF16, kind="ExternalInput")
    t_b = nc.dram_tensor("bias", [D, 1], F32, kind="ExternalInput")
    t_ia = nc.dram_tensor("idxa", [128, SA * 8], I16, kind="ExternalInput")
    t_ib = nc.dram_tensor("idxb", [128, SBp * 8], I16, kind="ExternalInput")
    t_dd = nc.dram_tensor("dstt", [CHUNK, ncalls * 128], BF16, kind="ExternalInput")
    t_ones = nc.dram_tensor("ones", [CHUNK, CHUNK * 128], BF16, kind="ExternalInput")
    t_iota = nc.dram_tensor("iota", [128, CHUNK * WIN], BF16, kind="ExternalInput")
    t_rec = nc.dram_tensor("rec", [WIN, N_WIN], F32, kind="ExternalInput")
    t_id = nc.dram_tensor("ident", [WIN, WIN], BF16, kind="ExternalInput")
    t_out = nc.dram_tensor("out", [D, NPC], F32, kind="ExternalOutput")

    ncallsA = -(-SA // CHUNK)

    NSLICE = 6

    with tile.TileContext(nc) as tc:
        with (
            tc.tile_pool(name="const", bufs=1) as cpool,
            tc.tile_pool(name="idx", bufs=1) as ipool,
            tc.tile_pool(name="msgsa", bufs=5) as mpa,
            tc.tile_pool(name="msgsb", bufs=4) as mpb,
            tc.tile_pool(name="oha", bufs=5) as opa,
            tc.tile_pool(name="ohb", bufs=4) as opb,
            tc.tile_pool(name="norm", bufs=4) as npool,
            tc.tile_pool(name="hpo", bufs=2) as hpool,
            tc.tile_pool(name="repb", bufs=3) as rep_pool,
            tc.tile_pool(name="psacc", bufs=3, space="PSUM") as ps_acc,
            tc.tile_pool(name="pstr", bufs=2, space="PSUM") as ps_tr,
            tc.tile_pool(name="psz", bufs=1, space="PSUM") as ps_z,
            tc.tile_pool(name="psrep", bufs=2, space="PSUM") as ps_rep,
        ):
            # ---- constants (iota / identity supplied from host) ----
            ident = cpool.tile([WIN, WIN], BF16)
            nc.sync.dma_start(out=ident[:], in_=t_id[:])
            wt_sb = cpool.tile([D, D], BF16)
            nc.sync.dma_start(out=wt_sb[:], in_=t_wt[:])
            b_sb = cpool.tile([D, 1], F32)
            nc.sync.dma_start(out=b_sb[:], in_=t_b[:])
            iota_f = cpool.tile([128, CHUNK * WIN], BF16)
            nc.sync.dma_start(out=iota_f[:], in_=t_iota[:])
            rec_sb = cpool.tile([WIN, N_WIN], F32)
            nc.sync.dma_start(out=rec_sb[:], in_=t_rec[:])
            ones_sb = cpool.tile([CHUNK, CHUNK * 128], BF16)
            nc.sync.dma_start(out=ones_sb[:], in_=t_ones[:])

            # index / dst tables, loaded in slices so the first gathers
            # can start before the whole tables arrive
            # warmup: a minimal gather issued first so the Q7 ext-isa
            # IRAM load (~6us) overlaps the index-table DMAs; the zero
            # indices come from a local memset, not a DMA
            wu_idx = cpool.tile([128, 8], I16)
            nc.vector.memset(wu_idx[:], 0)
            wu_out = cpool.tile([128, 1, ROW], BF16)
            nc.gpsimd.dma_gather(
                wu_out[:, :1, :], t_xlo[:], wu_idx[:, :8], 128, 128, ROW,
                single_packet=False, queue_num=0,
            )

            ia_sb = ipool.tile([128, SA * 8], I16)
            ib_sb = ipool.tile([128, SBp * 8], I16)
            dd_sb = ipool.tile([CHUNK, ncalls * 128], BF16)

            def sliced_load(sb, t, width, unit):
                n = -(-width // unit)
                first = max(1, n // 16) * unit
                per = -(-(width - first) // (NSLICE * unit)) * unit
                bounds = [0, first]
                while bounds[-1] < width:
                    bounds.append(min(bounds[-1] + per, width))
                for a, b in zip(bounds, bounds[1:]):
                    nc.sync.dma_start(out=sb[:, a:b], in_=t[:, a:b])

            sliced_load(ia_sb, t_ia, SA * 8, 8)
            sliced_load(ib_sb, t_ib, SBp * 8, 8)
            sliced_load(dd_sb, t_dd, ncalls * 128, 128)

            out_sb = cpool.tile([D, NPC], F32)

            call_tiles = {}
            cursor = [0, 0]

            def emit_call(i):
                st, pos, nsub = calls[i]
                if st == 0:
                    mp, op, tsrc, isb = mpa, opa, t_xlo, ia_sb
                else:
                    mp, op, tsrc, isb = mpb, opb, t_xhi, ib_sb
                msgs = mp.tile([128, CHUNK, ROW], BF16)
                nidx = nsub * 128
                # per-packet mode: one SDMA packet per descriptor.
                # single_packet=True measures the same speed here but
                # intermittently corrupts gathers; per-packet is the
                # reliable mode.  Padding rows gather the zero row.
                nc.gpsimd.dma_gather(
                    msgs[:, :nsub, :],
                    tsrc[:],
                    isb[:, pos * 8 : pos * 8 + nsub * 8],
                    nidx,
                    nidx,
                    ROW,
                    single_packet=False,
                    queue_num=i % NQ,
                )
                # Expand this call's dst values along the free dim with
                # K=CHUNK PE matmuls (lhsT = dst chunk [CHUNK subtiles, 128
                # lanes], rhs = block-identity ones -> rep[p, s*128+w] =
                # dst(s, p)), cast f32 PSUM -> bf16 SBUF on the idle ACT
                # engine, then compare on DVE.  The packed bf16 replica
                # (instead of a stride-0 broadcast operand) keeps the
                # is_equal in the 2x single-port DVE mode, which also
                # avoids starving SWDGE descriptor writes.
                rep_bf = rep_pool.tile([128, CHUNK * WIN], BF16)
                half = 512  # one PSUM bank of f32 per matmul
                for h in range(CHUNK * 128 // half):
                    c0 = h * half
                    n_h = min(nsub * 128 - c0, half)
                    if n_h <= 0:
                        break
                    rep = ps_rep.tile([128, half], F32)
                    nc.tensor.matmul(
                        out=rep[:, :n_h],
                        lhsT=dd_sb[:, i * 128 : (i + 1) * 128],
                        rhs=ones_sb[:, c0 : c0 + n_h],
                        start=True,
                        stop=True,
                    )
                    nc.scalar.activation(
                        out=rep_bf[:, c0 : c0 + n_h],
                        in_=rep[:, :n_h],
                        func=mybir.ActivationFunctionType.Copy,
                    )
                oh = op.tile([128, CHUNK * WIN], BF16)
                nc.vector.tensor_tensor(
                    out=oh[:, : nsub * WIN],
                    in0=iota_f[:, : nsub * WIN],
                    in1=rep_bf[:, : nsub * WIN],
                    op=mybir.AluOpType.is_equal,
                )
                call_tiles[i] = (msgs, oh)

            def tiles_for(st, s):
                k = s // CHUNK
                while cursor[st] <= k:
                    emit_call(cursor[st] if st == 0 else ncallsA + cursor[st])
                    cursor[st] += 1
                i = k if st == 0 else ncallsA + k
                msgs, oh = call_tiles[i]
                return msgs, oh, s % CHUNK

            OUT_PIECES = 4
            opiece = [0]

            for w in range(N_WIN):
                subs = [(0, int(offA[w]) + j) for j in range(int(kA[w]))]
                subs += [(1, int(offB[w]) + j) for j in range(int(kB[w]))]
                ps = ps_acc.tile([WIN, D], F32)
                for j, (st, s) in enumerate(subs):
                    msgs, oh, col = tiles_for(st, s)
                    nc.tensor.matmul(
                        out=ps[:],
                        lhsT=oh[:, col * WIN : (col + 1) * WIN],
                        rhs=msgs[:, col, :D],
                        start=(j == 0),
                        stop=(j == len(subs) - 1),
                    )
                h_w = npool.tile([WIN, D], BF16)
                nc.scalar.activation(
                    out=h_w[:],
                    in_=ps[:],
                    func=mybir.ActivationFunctionType.Copy,
                    scale=rec_sb[:, w : w + 1],
                )
                pst = ps_tr.tile([D, WIN], BF16)
                nc.tensor.transpose(out=pst[:], in_=h_w[:], identity=ident[:])
                ht = hpool.tile([D, WIN], BF16)
                nc.scalar.activation(
                    out=ht[:], in_=pst[:], func=mybir.ActivationFunctionType.Copy
                )
                z = ps_z.tile([D, WIN], F32)
                nc.tensor.matmul(
                    out=z[:], lhsT=wt_sb[:], rhs=ht[:], start=True, stop=True
                )
                t0 = w * WIN
                nc.scalar.activation(
                    out=out_sb[:, t0 : t0 + WIN],
                    in_=z[:],
                    func=mybir.ActivationFunctionType.Identity,
                    bias=b_sb[:, 0:1],
                )
                # stream the finished out columns back in pieces
                nxt = (opiece[0] + 1) * N_WIN // OUT_PIECES
                if w + 1 == nxt:
                    a = opiece[0] * N_WIN // OUT_PIECES * WIN
                    bnd = nxt * WIN
                    nc.sync.dma_start(out=t_out[:, a:bnd], in_=out_sb[:, a:bnd])
                    opiece[0] += 1

    nc.compile()
    return nc


def kernel(x, src, dst, W, b):
    x = np.asarray(x, dtype=np.float32)
    W = np.asarray(W, dtype=np.float32)
    b = np.asarray(b, dtype=np.float32)

    (xlo, xhi, kA, kB, SA, SBp, offA, offB, calls, per_core, assign) = _prep(
        x, src, dst
    )
    nc = _build_program(kA, kB, SA, SBp, offA, offB, calls)

    wt = np.ascontiguousarray(W.T.astype(BF))
    bcol = np.ascontiguousarray(b.reshape(D, 1))
    iota_arr = np.tile(
        np.arange(WIN, dtype=BF)[None, :], (128, CHUNK)
    ).copy()
    ident_arr = np.eye(WIN, dtype=BF)
    ones_arr = np.zeros((CHUNK, CHUNK * 128), dtype=BF)
    for s in range(CHUNK):
        ones_arr[s, s * 128 : (s + 1) * 128] = 1.0

    in_maps = []
    for c in range(N_CORES):
        iA, dA, iB, dB, cnts, rec_c = per_core[c]
        in_maps.append(
            {
                "xlo": xlo,
                "xhi": xhi,
                "wt": wt,
                "bias": bcol,
                "idxa": _wrap_idx(iA),
                "idxb": _wrap_idx(iB),
                "dstt": _wrap_dst(dA, dB, calls),
                "iota": iota_arr,
                "rec": rec_c,
                "ident": ident_arr,
                "ones": ones_arr,
            }
        )

    res = run_bass_kernel_spmd(nc, in_maps, list(range(N_CORES)))
    LAST["results"] = res
    LAST["exec_time_ns"] = res.exec_time_ns

    # Undo the balanced window permutation: core c slot j holds global
    # window assign[c, j] as out columns [j*WIN, (j+1)*WIN).
    full = np.empty((N_GWIN * WIN, D), dtype=np.float32)
    for c in range(N_CORES):
        out_c = res.results[c]["out"]  # [D, NPC]
        for j in range(N_WIN):
            g = int(assign[c, j])
            full[g * WIN : (g + 1) * WIN] = out_c[:, j * WIN : (j + 1) * WIN].T
    return np.ascontiguousarray(full[:N_NODES])
